# revision 3
# baseline (speedup 1.0000x reference)
"""Cumulative (causal) normalization for TRN2, 8 NeuronCores.

x: [32, 512, 4000] f32.  out = (x - cum_mean) / sqrt(cum_var + eps), cumsum
along frames.  Data parallel: 16384 rows split 2048/core, 16 row-tiles of
[128, 4000] per core.  bf16 I/O (host casts), f32 in-engine state.

Per tile, three fused instructions via custom DVE ops (in-body prefix scans
run at ~1 elem/cycle vs 2.05 for the stock TensorTensorScan):

  W   = n*(cumsum(x^2) + eps) - cumsum(x)^2     [DVE custom CUMW,  f32 out]
  r   = 1/sqrt(|W|)                             [ACT Abs_reciprocal_sqrt]
  out = (n*x - cumsum(x)) * r                   [DVE custom CUMOUT, bf16]

with n = k+1 generated in-body by scan(ADD, One, init=Zero).  This is the
reference math scaled by n: num/den = n(x-mean) / (n*sqrt(var+eps)); eps
enters as cumsum-seed so W(k=0) = eps exactly (no NaN at the first frame).
"""

import numpy as np
import ml_dtypes

EPS = 1e-4
B, NBINS, F = 32, 512, 4000
P = 128
NCORES = 8
ROWS = B * NBINS                 # 16384
ROWS_PER_CORE = ROWS // NCORES   # 2048
NT = ROWS_PER_CORE // P          # 16

_CACHE = {}


def _ref_cumout(in0, in1, s0, s1, imm2):
    x = in0.astype(np.float32).reshape(in0.shape[0], -1)
    r = in1.astype(np.float32).reshape(in0.shape[0], -1)
    n = np.arange(1, x.shape[-1] + 1, dtype=np.float32)
    num = n[None, :] * x - np.cumsum(x.astype(np.float64), -1).astype(np.float32)
    return num * r


def _ref_cumw(in0, in1, s0, s1, imm2):
    x = in0.astype(np.float32).reshape(in0.shape[0], -1)
    n = np.arange(1, x.shape[-1] + 1, dtype=np.float32)
    s2e = np.cumsum((x * x).astype(np.float64), -1).astype(np.float32) + s0
    s1c = np.cumsum(x.astype(np.float64), -1).astype(np.float32)
    return n[None, :] * s2e - s1c * s1c


def _register_dve_ops():
    """Register the two fused ops in concourse's custom-DVE catalog (the
    documented extension point: append DveOp to dve_ops.OPS).  uops_sha is
    computed at runtime so the pin always matches this build's lower()."""
    from concourse import dve_ops
    from concourse.dve_ops import has_src1
    from concourse.dve_spec import (
        Spec, Src0, Src1, C0, One, Zero, sq, lower, AluOp, scan,
    )
    from concourse.dve_uop import DveOpSpec

    made = {}

    def _mk(name, spec):
        if name in dve_ops._SUB_OPCODE_FOR_NAME:
            for op in dve_ops.OPS:
                if op.name == name:
                    made[name] = op
                    return
        row = max(dve_ops._SUB_OPCODE_FOR_NAME.values()) + 1
        assert row < 0x20, "custom DVE row overflow"
        op = dve_ops.DveOp(name, spec, subdim=False, uops_sha={})
        dve_ops.OPS.append(op)
        dve_ops._SUB_OPCODE_FOR_NAME[name] = row
        dve_ops.CUSTOM_DVE_SPECS[name] = spec
        for ver in ("v3", "v4"):
            tmp = DveOpSpec(name=name, opcode=row,
                            uops=lower(spec, ver=ver),
                            rd1_en=has_src1(spec))
            op.uops_sha[ver] = tmp.sha(ver)
        made[name] = op

    n_ = scan(AluOp.ADD, One, init=Zero)          # n = k+1
    _mk("CUMW_ANT",
        Spec(body=n_ * scan(AluOp.ADD, sq(Src0), init=C0)
                  - sq(scan(AluOp.ADD, Src0)),
             reference=_ref_cumw))
    _mk("CUMOUT_ANT",
        Spec(body=(n_ * Src0 - scan(AluOp.ADD, Src0)) * Src1,
             reference=_ref_cumout))
    return made["CUMW_ANT"], made["CUMOUT_ANT"]


def _build():
    import concourse.bacc as bacc
    import concourse.mybir as mybir
    import concourse.tile as tile

    WOP, OUTOP = _register_dve_ops()

    f32 = mybir.dt.float32
    bf16 = mybir.dt.bfloat16
    ARS = mybir.ActivationFunctionType.Abs_reciprocal_sqrt

    nc = bacc.Bacc()
    x_d = nc.dram_tensor("x", [ROWS_PER_CORE, F], bf16, kind="ExternalInput")
    o_d = nc.dram_tensor("out", [ROWS_PER_CORE, F], bf16, kind="ExternalOutput")

    with tile.TileContext(nc) as tc:
        with (
            tc.tile_pool(name="io", bufs=4) as io,
            tc.tile_pool(name="io2", bufs=4) as io2,
            tc.tile_pool(name="ww", bufs=3) as ww,
            tc.tile_pool(name="wr", bufs=3) as wr,
        ):
            for it in range(NT):
                r0 = it * P
                x_t = io.tile([P, F], bf16, tag="x")
                nc.sync.dma_start(out=x_t, in_=x_d[r0:r0 + P, :])
                out_t = io2.tile([P, F], bf16, tag="o")

                Wt = ww.tile([P, F], f32, tag="W")
                nc.vector._custom_dve(WOP, out=Wt, in0=x_t, s0=EPS)

                r_t = wr.tile([P, F], bf16, tag="r")
                nc.scalar.activation(r_t, Wt, ARS)

                nc.vector._custom_dve(OUTOP, out=out_t, in0=x_t, in1=r_t)

                nc.sync.dma_start(out=o_d[r0:r0 + P, :], in_=out_t)

    nc.finalize()
    return nc


def kernel(x: np.ndarray) -> np.ndarray:
    from concourse import bass_utils

    assert x.shape == (B, NBINS, F) and x.dtype == np.float32
    if "nc" not in _CACHE:
        _CACHE["nc"] = _build()
    nc = _CACHE["nc"]

    xb = np.ascontiguousarray(x.reshape(ROWS, F)).astype(ml_dtypes.bfloat16)
    in_maps = [
        {"x": xb[c * ROWS_PER_CORE:(c + 1) * ROWS_PER_CORE]}
        for c in range(NCORES)
    ]
    res = bass_utils.run_bass_kernel_spmd(nc, in_maps,
                                          core_ids=list(range(NCORES)))
    out = np.concatenate([r["out"] for r in res.results], axis=0)
    return out.astype(np.float32).reshape(B, NBINS, F)


# revision 4
# speedup vs baseline: 1.0010x; 1.0010x over previous
"""Cumulative (causal) normalization for TRN2, 8 NeuronCores.

x: [32, 512, 4000] f32.  out = (x - cum_mean) / sqrt(cum_var + eps), cumsum
along frames.  Data parallel: 16384 rows split 2048/core, 16 row-tiles of
[128, 4000] per core.  bf16 I/O (host casts), f32 in-engine state.

Per tile, three fused instructions via custom DVE ops (in-body prefix scans
run at ~1 elem/cycle vs 2.05 for the stock TensorTensorScan):

  W   = n*(cumsum(x^2) + eps) - cumsum(x)^2     [DVE custom CUMW,  f32 out]
  r   = 1/sqrt(|W|)                             [ACT Abs_reciprocal_sqrt]
  out = (n*x - cumsum(x)) * r                   [DVE custom CUMOUT, bf16]

with n = k+1 generated in-body by scan(ADD, One, init=Zero).  This is the
reference math scaled by n: num/den = n(x-mean) / (n*sqrt(var+eps)); eps
enters as cumsum-seed so W(k=0) = eps exactly (no NaN at the first frame).
"""

import numpy as np
import ml_dtypes

EPS = 1e-4
B, NBINS, F = 32, 512, 4000
P = 128
NCORES = 8
ROWS = B * NBINS                 # 16384
ROWS_PER_CORE = ROWS // NCORES   # 2048
NT = ROWS_PER_CORE // P          # 16

_CACHE = {}


def _ref_cumout(in0, in1, s0, s1, imm2):
    x = in0.astype(np.float32).reshape(in0.shape[0], -1)
    r = in1.astype(np.float32).reshape(in0.shape[0], -1)
    n = np.arange(1, x.shape[-1] + 1, dtype=np.float32)
    num = n[None, :] * x - np.cumsum(x.astype(np.float64), -1).astype(np.float32)
    return num * r


def _ref_cumw(in0, in1, s0, s1, imm2):
    x = in0.astype(np.float32).reshape(in0.shape[0], -1)
    n = np.arange(1, x.shape[-1] + 1, dtype=np.float32)
    s2e = np.cumsum((x * x).astype(np.float64), -1).astype(np.float32) + s0
    s1c = np.cumsum(x.astype(np.float64), -1).astype(np.float32)
    return n[None, :] * s2e - s1c * s1c


def _register_dve_ops():
    """Register the two fused ops in concourse's custom-DVE catalog (the
    documented extension point: append DveOp to dve_ops.OPS).  uops_sha is
    computed at runtime so the pin always matches this build's lower()."""
    from concourse import dve_ops
    from concourse.dve_ops import has_src1
    from concourse.dve_spec import (
        Spec, Src0, Src1, C0, One, Zero, sq, lower, AluOp, scan,
    )
    from concourse.dve_uop import DveOpSpec

    made = {}

    def _mk(name, spec):
        if name in dve_ops._SUB_OPCODE_FOR_NAME:
            for op in dve_ops.OPS:
                if op.name == name:
                    made[name] = op
                    return
        row = max(dve_ops._SUB_OPCODE_FOR_NAME.values()) + 1
        assert row < 0x20, "custom DVE row overflow"
        op = dve_ops.DveOp(name, spec, subdim=False, uops_sha={})
        dve_ops.OPS.append(op)
        dve_ops._SUB_OPCODE_FOR_NAME[name] = row
        dve_ops.CUSTOM_DVE_SPECS[name] = spec
        for ver in ("v3", "v4"):
            tmp = DveOpSpec(name=name, opcode=row,
                            uops=lower(spec, ver=ver),
                            rd1_en=has_src1(spec))
            op.uops_sha[ver] = tmp.sha(ver)
        made[name] = op

    n_ = scan(AluOp.ADD, One, init=Zero)          # n = k+1
    _mk("CUMW_ANT",
        Spec(body=n_ * scan(AluOp.ADD, sq(Src0), init=C0)
                  - sq(scan(AluOp.ADD, Src0)),
             reference=_ref_cumw))
    _mk("CUMOUT_ANT",
        Spec(body=(n_ * Src0 - scan(AluOp.ADD, Src0)) * Src1,
             reference=_ref_cumout))
    return made["CUMW_ANT"], made["CUMOUT_ANT"]


def _build():
    import concourse.bacc as bacc
    import concourse.mybir as mybir
    import concourse.tile as tile

    WOP, OUTOP = _register_dve_ops()

    f32 = mybir.dt.float32
    bf16 = mybir.dt.bfloat16
    ARS = mybir.ActivationFunctionType.Abs_reciprocal_sqrt

    nc = bacc.Bacc()
    x_d = nc.dram_tensor("x", [ROWS_PER_CORE, F], bf16, kind="ExternalInput")
    o_d = nc.dram_tensor("out", [ROWS_PER_CORE, F], bf16, kind="ExternalOutput")

    with tile.TileContext(nc) as tc:
        with (
            tc.tile_pool(name="io", bufs=3) as io,
            tc.tile_pool(name="io2", bufs=3) as io2,
            tc.tile_pool(name="ww", bufs=3) as ww,
            tc.tile_pool(name="wr", bufs=3) as wr,
        ):
            for it in range(NT):
                r0 = it * P
                x_t = io.tile([P, F], bf16, tag="x")
                nc.sync.dma_start(out=x_t, in_=x_d[r0:r0 + P, :])
                out_t = io2.tile([P, F], bf16, tag="o")

                Wt = ww.tile([P, F], f32, tag="W")
                nc.vector._custom_dve(WOP, out=Wt, in0=x_t, s0=EPS)

                r_t = wr.tile([P, F], bf16, tag="r")
                nc.scalar.activation(r_t, Wt, ARS)

                nc.vector._custom_dve(OUTOP, out=out_t, in0=x_t, in1=r_t)

                nc.sync.dma_start(out=o_d[r0:r0 + P, :], in_=out_t)

    nc.finalize()
    return nc


def kernel(x: np.ndarray) -> np.ndarray:
    from concourse import bass_utils

    assert x.shape == (B, NBINS, F) and x.dtype == np.float32
    if "nc" not in _CACHE:
        _CACHE["nc"] = _build()
    nc = _CACHE["nc"]

    xb = np.ascontiguousarray(x.reshape(ROWS, F)).astype(ml_dtypes.bfloat16)
    in_maps = [
        {"x": xb[c * ROWS_PER_CORE:(c + 1) * ROWS_PER_CORE]}
        for c in range(NCORES)
    ]
    res = bass_utils.run_bass_kernel_spmd(nc, in_maps,
                                          core_ids=list(range(NCORES)))
    out = np.concatenate([r["out"] for r in res.results], axis=0)
    return out.astype(np.float32).reshape(B, NBINS, F)
